# revision 7
# baseline (speedup 1.0000x reference)
"""Trainium2 Bass kernel for a dense transformer block (pre-LN, 8-head causal
attention + FFN), data-parallel over batch across 8 NeuronCores.

Reference computation (per token batch B=128, T=256, C=384, H=8, HS=48):
    h  = LN(x; g1, beta1)
    q,k,v = per-head projections of h
    attn  = causal-softmax(q k^T / sqrt(HS)) v      (concat heads)
    x1 = x + attn @ Wproj + bproj
    h2 = LN(x1; g2, beta2)
    out = x1 + relu(h2 @ W1 + b1) @ W2 + b2

Sharding: batch 128 -> 16 sequences per core; all parameters replicated.

On-chip design notes:
  * All matmul operands fp16 (PE runs 1 cycle/row, same as bf16, with 10-bit
    mantissa), accumulation fp32 in PSUM. Activations/residuals fp32.
  * LN affine (g, beta) is folded into the following weight matrices on the
    host: W' = diag(g) W, plus a rank-1 (ones x row) matmul for beta@W and
    biases, accumulated directly in PSUM.
  * Q/K stored head-transposed [d, t] with heads padded to 64 rows so each
    128-row tile holds two heads (offsets 0 and 64); score matmuls for the
    two heads run concurrently in distinct PE row groups.
  * Scores are computed transposed [s, t] so softmax sums reduce over the
    partition dim via matmul against an appended ones-column of V
    (V is stored augmented: 8 heads x (48 dims + ones col) = 392 cols).
  * Causal mask is applied post-exp as a 0/1 fp16 multiply on the two
    diagonal [128,128] blocks only; off-diagonal blocks need no mask and
    the fully-masked block is never computed.
  * Layout changes ([t,c] <-> [c,t]) ride the DMA xbar transpose (fp16).
"""

import numpy as np

import concourse.bass as bass
import concourse.mybir as mybir
import concourse.tile as tile
from concourse import bacc
from concourse.bass_utils import run_bass_kernel_spmd

F32 = mybir.dt.float32
F16 = mybir.dt.float16

# Model dims
B, T, C = 128, 256, 384
H, HS = 8, 48
FF = 4 * C           # 1536
EPS = 1e-5

# Sharding / tiling
NCORES = 8
NB = B // NCORES     # 16 sequences per core
TOK = NB * T         # 4096 tokens per core
P = 128
CCH = C // P         # 3 c-chunks
FCH = FF // P        # 12 ffn chunks
DPAD = 512           # q/k head-padded dim (4 tiles x 2 heads x 64)
QMT = DPAD // P      # 4
VW = H * (HS + 1)    # 392 augmented v width
GT = 512             # tokens per group (2 sequences)
NG = TOK // GT       # 8 groups
ISCALE = float(HS) ** -0.5


def _build_program():
    nc = bacc.Bacc(None, target_bir_lowering=False, debug=False)

    x_d = nc.dram_tensor("x", [TOK, C], F32, kind="ExternalInput").ap()
    wq_d = nc.dram_tensor("wq", [CCH, P, DPAD], F16, kind="ExternalInput").ap()
    wk_d = nc.dram_tensor("wk", [CCH, P, DPAD], F16, kind="ExternalInput").ap()
    wv_d = nc.dram_tensor("wv", [CCH, P, VW], F16, kind="ExternalInput").ap()
    wp_d = nc.dram_tensor("wp", [CCH, P, C], F16, kind="ExternalInput").ap()
    w1_d = nc.dram_tensor("w1", [CCH, P, FF], F16, kind="ExternalInput").ap()
    w2_d = nc.dram_tensor("w2", [FCH, P, C], F16, kind="ExternalInput").ap()
    rowq_d = nc.dram_tensor("rowq", [1, DPAD], F16, kind="ExternalInput").ap()
    rowk_d = nc.dram_tensor("rowk", [1, DPAD], F16, kind="ExternalInput").ap()
    rowv_d = nc.dram_tensor("rowv", [1, VW], F16, kind="ExternalInput").ap()
    rowp_d = nc.dram_tensor("rowp", [1, C], F16, kind="ExternalInput").ap()
    rowl_d = nc.dram_tensor("rowl", [1, C], F16, kind="ExternalInput").ap()
    b1t_d = nc.dram_tensor("b1t", [P, FCH], F32, kind="ExternalInput").ap()
    mask_d = nc.dram_tensor("maskmul", [P, P], F16, kind="ExternalInput").ap()
    out_d = nc.dram_tensor("out", [TOK, C], F32, kind="ExternalOutput").ap()

    with tile.TileContext(nc) as tc:
        _emit(nc, tc, x_d, wq_d, wk_d, wv_d, wp_d, w1_d, w2_d,
              rowq_d, rowk_d, rowv_d, rowp_d, rowl_d, b1t_d, mask_d, out_d)
    nc.compile()
    return nc


def _emit(nc, tc, x_d, wq_d, wk_d, wv_d, wp_d, w1_d, w2_d,
          rowq_d, rowk_d, rowv_d, rowp_d, rowl_d, b1t_d, mask_d, out_d):
    from contextlib import ExitStack
    with ExitStack() as ctx:
        const = ctx.enter_context(tc.tile_pool(name="const", bufs=1))
        big = ctx.enter_context(tc.tile_pool(name="big", bufs=1))
        ln = ctx.enter_context(tc.tile_pool(name="ln", bufs=4))
        grp = ctx.enter_context(tc.tile_pool(name="grp", bufs=2))
        att = ctx.enter_context(tc.tile_pool(name="att", bufs=4))
        outp = ctx.enter_context(tc.tile_pool(name="outp", bufs=3))
        psum = ctx.enter_context(tc.tile_pool(name="psum", bufs=8, space="PSUM"))

        def ps_tile():
            return psum.tile([P, 512], F32, name="ps", tag="ps")

        # ---- constants ----
        wq_sb = const.tile([P, CCH, DPAD], F16)
        wk_sb = const.tile([P, CCH, DPAD], F16)
        wv_sb = const.tile([P, CCH, VW], F16)
        wp_sb = const.tile([P, CCH, C], F16)
        w1_sb = const.tile([P, CCH, FF], F16)
        w2_sb = const.tile([P, FCH, C], F16)
        for cc in range(CCH):
            nc.sync.dma_start(wq_sb[:, cc, :], wq_d[cc])
            nc.sync.dma_start(wk_sb[:, cc, :], wk_d[cc])
            nc.sync.dma_start(wv_sb[:, cc, :], wv_d[cc])
            nc.sync.dma_start(wp_sb[:, cc, :], wp_d[cc])
            nc.sync.dma_start(w1_sb[:, cc, :], w1_d[cc])
        for fc in range(FCH):
            nc.sync.dma_start(w2_sb[:, fc, :], w2_d[fc])
        rowq_sb = const.tile([1, DPAD], F16)
        rowk_sb = const.tile([1, DPAD], F16)
        rowv_sb = const.tile([1, VW], F16)
        rowp_sb = const.tile([1, C], F16)
        rowl_sb = const.tile([1, C], F16)
        ones_sb = const.tile([1, GT], F16)
        b1t_sb = const.tile([P, FCH], F32)
        mask_sb = const.tile([P, P], F16)
        eps_sb = const.tile([P, 1], F32)
        nc.sync.dma_start(rowq_sb, rowq_d)
        nc.sync.dma_start(rowk_sb, rowk_d)
        nc.sync.dma_start(rowv_sb, rowv_d)
        nc.sync.dma_start(rowp_sb, rowp_d)
        nc.sync.dma_start(rowl_sb, rowl_d)
        nc.sync.dma_start(b1t_sb, b1t_d)
        nc.sync.dma_start(mask_sb, mask_d)
        nc.vector.memset(ones_sb, 1.0)
        nc.vector.memset(eps_sb, EPS)

        # ---- residual stream (fp32, resident) and transposed LN1 output ----
        x_all = big.tile([P, TOK // P, C], F32)
        hT = big.tile([P, CCH, TOK], F16)

        def layernorm_tile(src, dst_f16):
            """dst_f16 <- (src - mean) * rsqrt(var + eps), rows = tokens."""
            stats = ln.tile([P, 6], F32, tag="stats")
            mv = ln.tile([P, 2], F32, tag="mv")
            rstd = ln.tile([P, 1], F32, tag="rstd")
            nc.vector.bn_stats(out=stats, in_=src)
            nc.vector.bn_aggr(out=mv, in_=stats)
            nc.scalar.activation(out=rstd, in_=mv[:, 1:2],
                                 func=mybir.ActivationFunctionType.Sqrt,
                                 bias=eps_sb, scale=1.0)
            nc.vector.reciprocal(out=rstd, in_=rstd)
            nc.vector.tensor_scalar(out=dst_f16, in0=src,
                                    scalar1=mv[:, 0:1], scalar2=rstd,
                                    op0=mybir.AluOpType.subtract,
                                    op1=mybir.AluOpType.mult)

        # ---- phase 1: load x, LN1, transpose h into hT ----
        for it in range(TOK // P):
            nc.sync.dma_start(x_all[:, it, :], x_d[it * P:(it + 1) * P, :])
            hN = ln.tile([P, C], F16, tag="hN")
            layernorm_tile(x_all[:, it, :], hN)
            for cc in range(CCH):
                nc.sync.dma_start(hT[:, cc, it * P:(it + 1) * P],
                                  hN[:, cc * P:(cc + 1) * P], transpose=True)

        # ---- phases 2-4 per group of 2 sequences ----
        for g in range(NG):
            t0 = g * GT
            qT = grp.tile([P, QMT, GT], F16, tag="qT")
            kT = grp.tile([P, QMT, GT], F16, tag="kT")
            vaug = grp.tile([P, GT // P, VW], F16, tag="vaug")
            attnT = grp.tile([P, CCH, GT], F16, tag="attnT")

            # Q/K projections, head-transposed+padded
            for dst, w_sb, row_sb in ((qT, wq_sb, rowq_sb), (kT, wk_sb, rowk_sb)):
                for m in range(QMT):
                    ps = ps_tile()
                    for cc in range(CCH):
                        nc.tensor.matmul(ps, lhsT=w_sb[:, cc, m * P:(m + 1) * P],
                                         rhs=hT[:, cc, t0:t0 + GT],
                                         start=(cc == 0), stop=False)
                    nc.tensor.matmul(ps, lhsT=row_sb[:, m * P:(m + 1) * P],
                                     rhs=ones_sb, start=False, stop=True)
                    nc.scalar.copy(dst[:, m, :], ps)

            # V (natural layout, augmented with ones column per head)
            for st in range(GT // P):
                ps = ps_tile()
                for cc in range(CCH):
                    nc.tensor.matmul(ps[:, :VW],
                                     lhsT=hT[:, cc, t0 + st * P:t0 + (st + 1) * P],
                                     rhs=wv_sb[:, cc, :],
                                     start=(cc == 0), stop=False)
                nc.tensor.matmul(ps[:, :VW], lhsT=ones_sb[:, :P], rhs=rowv_sb,
                                 start=False, stop=True)
                nc.vector.tensor_copy(vaug[:, st, :], ps[:, :VW])

            # attention per sequence in group
            for b2 in range(2):
                s0 = b2 * T   # token offset of this seq inside the group
                aps0 = ps_tile()   # attn numerator+denominator, t-chunk 0
                aps1 = ps_tile()   # t-chunk 1
                for m in range(QMT):
                    for hh in range(2):
                        h = 2 * m + hh
                        off = 64 * hh
                        # transposed scores [s, t] for this head
                        sp0 = ps_tile()
                        nc.tensor.matmul(
                            sp0[:, :T],
                            lhsT=kT[off:off + HS, m, s0:s0 + P],
                            rhs=qT[off:off + HS, m, s0:s0 + T],
                            start=True, stop=True)
                        sp1 = ps_tile()
                        nc.tensor.matmul(
                            sp1[:, :P],
                            lhsT=kT[off:off + HS, m, s0 + P:s0 + T],
                            rhs=qT[off:off + HS, m, s0 + P:s0 + T],
                            start=True, stop=True)
                        ew0 = att.tile([P, T], F16, tag="ew0")
                        ew1 = att.tile([P, P], F16, tag="ew1")
                        nc.scalar.activation(out=ew0, in_=sp0[:, :T],
                                             func=mybir.ActivationFunctionType.Exp,
                                             scale=ISCALE)
                        nc.scalar.activation(out=ew1, in_=sp1[:, :P],
                                             func=mybir.ActivationFunctionType.Exp,
                                             scale=ISCALE)
                        # causal mask on the two diagonal blocks
                        nc.vector.tensor_mul(ew0[:, :P], ew0[:, :P], mask_sb)
                        nc.vector.tensor_mul(ew1, ew1, mask_sb)
                        hs = h * (HS + 1)
                        # t-chunk 0 only sees s-chunk 0
                        nc.tensor.matmul(aps0[:, hs:hs + HS + 1],
                                         lhsT=ew0[:, :P],
                                         rhs=vaug[:, b2 * 2, hs:hs + HS + 1],
                                         start=True, stop=True)
                        # t-chunk 1 sees both s-chunks
                        nc.tensor.matmul(aps1[:, hs:hs + HS + 1],
                                         lhsT=ew0[:, P:T],
                                         rhs=vaug[:, b2 * 2, hs:hs + HS + 1],
                                         start=True, stop=False)
                        nc.tensor.matmul(aps1[:, hs:hs + HS + 1],
                                         lhsT=ew1,
                                         rhs=vaug[:, b2 * 2 + 1, hs:hs + HS + 1],
                                         start=False, stop=True)
                # normalize and transpose into attnT
                for tch, aps in ((0, aps0), (1, aps1)):
                    a3 = aps[:, :VW].rearrange("p (h w) -> p h w", w=HS + 1)
                    recips = att.tile([P, H], F32, tag="recips")
                    nc.vector.reciprocal(out=recips, in_=a3[:, :, HS])
                    attn_n = att.tile([P, C], F16, tag="attn_n")
                    for h in range(H):
                        nc.vector.tensor_scalar_mul(
                            out=attn_n[:, h * HS:(h + 1) * HS],
                            in0=a3[:, h, :HS], scalar1=recips[:, h:h + 1])
                    tt = b2 * 2 + tch
                    for cc in range(CCH):
                        nc.sync.dma_start(attnT[:, cc, tt * P:(tt + 1) * P],
                                          attn_n[:, cc * P:(cc + 1) * P],
                                          transpose=True)

            # output projection + residual (x1 overwrites x in place)
            for tt in range(GT // P):
                it = g * (GT // P) + tt
                ps = ps_tile()
                for cc in range(CCH):
                    nc.tensor.matmul(ps[:, :C],
                                     lhsT=attnT[:, cc, tt * P:(tt + 1) * P],
                                     rhs=wp_sb[:, cc, :],
                                     start=(cc == 0), stop=False)
                nc.tensor.matmul(ps[:, :C], lhsT=ones_sb[:, :P], rhs=rowp_sb,
                                 start=False, stop=True)
                nc.vector.tensor_add(x_all[:, it, :], x_all[:, it, :], ps[:, :C])

        # ---- phases 5-7 per group: LN2 + FFN + residual ----
        for g in range(NG):
            h2T = grp.tile([P, CCH, GT], F16, tag="h2T")
            for tt in range(GT // P):
                it = g * (GT // P) + tt
                h2 = ln.tile([P, C], F16, tag="h2")
                layernorm_tile(x_all[:, it, :], h2)
                for cc in range(CCH):
                    nc.sync.dma_start(h2T[:, cc, tt * P:(tt + 1) * P],
                                      h2[:, cc * P:(cc + 1) * P], transpose=True)
            rg = grp.tile([P, FCH, GT], F16, tag="rg")
            for fc in range(FCH):
                ps = ps_tile()
                for cc in range(CCH):
                    nc.tensor.matmul(ps, lhsT=w1_sb[:, cc, fc * P:(fc + 1) * P],
                                     rhs=h2T[:, cc, :],
                                     start=(cc == 0), stop=(cc == CCH - 1))
                nc.scalar.activation(out=rg[:, fc, :], in_=ps,
                                     func=mybir.ActivationFunctionType.Relu,
                                     bias=b1t_sb[:, fc:fc + 1], scale=1.0)
            for tt in range(GT // P):
                it = g * (GT // P) + tt
                ps = ps_tile()
                for fc in range(FCH):
                    nc.tensor.matmul(ps[:, :C],
                                     lhsT=rg[:, fc, tt * P:(tt + 1) * P],
                                     rhs=w2_sb[:, fc, :],
                                     start=(fc == 0), stop=False)
                nc.tensor.matmul(ps[:, :C], lhsT=ones_sb[:, :P], rhs=rowl_sb,
                                 start=False, stop=True)
                ot = outp.tile([P, C], F32, tag="ot")
                nc.vector.tensor_add(ot, x_all[:, it, :], ps[:, :C])
                nc.sync.dma_start(out_d[it * P:(it + 1) * P, :], ot)


def _prep_weights(Wq, Wk, Wv, Wproj, bproj, W1, b1, W2, b2, g1, beta1, g2, beta2):
    f16 = np.float16
    g1 = g1.astype(np.float64)
    g2 = g2.astype(np.float64)

    def qk_pack(W):
        Ws = g1[None, :, None] * W.astype(np.float64)      # [H, C, HS]
        pad = np.zeros((CCH, P, DPAD), np.float64)
        row = np.zeros((1, DPAD), np.float64)
        # beta1 @ W uses the unscaled W: h_aff@W = h_norm@(g1*W) + beta1@W
        beta_r = np.einsum('c,hcd->hd', beta1.astype(np.float64),
                           W.astype(np.float64))
        for h in range(H):
            m, hh = divmod(h, 2)
            col = m * P + 64 * hh
            pad[:, :, col:col + HS] = Ws[h].reshape(CCH, P, HS)
            row[0, col:col + HS] = beta_r[h]
        return pad.astype(f16), row.astype(f16)

    wq_pad, rowq = qk_pack(Wq)
    wk_pad, rowk = qk_pack(Wk)

    Wvs = (g1[None, :, None] * Wv.astype(np.float64))       # [H, C, HS]
    beta_v = np.einsum('c,hcd->hd', beta1.astype(np.float64), Wv.astype(np.float64))
    wv_aug = np.zeros((CCH, P, VW), np.float64)
    rowv = np.zeros((1, VW), np.float64)
    for h in range(H):
        col = h * (HS + 1)
        wv_aug[:, :, col:col + HS] = Wvs[h].reshape(CCH, P, HS)
        rowv[0, col:col + HS] = beta_v[h]
        rowv[0, col + HS] = 1.0
    wv_aug = wv_aug.astype(f16)
    rowv = rowv.astype(f16)

    wp = Wproj.astype(f16).reshape(CCH, P, C)
    rowp = bproj.astype(f16).reshape(1, C)

    W1s = g2[:, None] * W1.astype(np.float64)
    w1p = W1s.astype(f16).reshape(CCH, P, FF)
    b1tot = (b1.astype(np.float64)
             + beta2.astype(np.float64) @ W1.astype(np.float64))
    b1t = b1tot.astype(np.float32).reshape(FCH, P).T.copy()   # [P, FCH]

    w2p = W2.astype(f16).reshape(FCH, P, C)
    rowl = b2.astype(f16).reshape(1, C)

    maskmul = np.triu(np.ones((P, P), f16))  # [s, t]: valid iff s <= t
    return dict(wq=wq_pad, wk=wk_pad, wv=wv_aug, wp=wp, w1=w1p, w2=w2p,
                rowq=rowq, rowk=rowk, rowv=rowv, rowp=rowp, rowl=rowl,
                b1t=b1t, maskmul=maskmul)


_CACHED = {}


def _get_program():
    if "nc" not in _CACHED:
        _CACHED["nc"] = _build_program()
    return _CACHED["nc"]


def _run(inputs, trace=False):
    x = np.asarray(inputs["x"], np.float32)
    wdict = _prep_weights(
        np.asarray(inputs["Wq"]), np.asarray(inputs["Wk"]),
        np.asarray(inputs["Wv"]), np.asarray(inputs["Wproj"]),
        np.asarray(inputs["bproj"]), np.asarray(inputs["W1"]),
        np.asarray(inputs["b1"]), np.asarray(inputs["W2"]),
        np.asarray(inputs["b2"]), np.asarray(inputs["g1"]),
        np.asarray(inputs["beta1"]), np.asarray(inputs["g2"]),
        np.asarray(inputs["beta2"]))

    shards = x.reshape(NCORES, NB * T, C)
    in_maps = [dict(wdict, x=np.ascontiguousarray(shards[i]))
               for i in range(NCORES)]
    nc = _get_program()
    res = run_bass_kernel_spmd(nc, in_maps, list(range(NCORES)), trace=trace)
    out = np.stack([res.results[i]["out"] for i in range(NCORES)])
    return out.reshape(B, T, C).astype(np.float32), res


def kernel(**inputs):
    out, _ = _run(inputs, trace=False)
    return out


# revision 10
# speedup vs baseline: 1.2304x; 1.2304x over previous
"""Trainium2 Bass kernel for a dense transformer block (pre-LN, 8-head causal
attention + FFN), data-parallel over batch across 8 NeuronCores.

Reference computation (B=128, T=256, C=384, H=8, HS=48):
    h  = LN(x; g1, beta1)
    q,k,v = per-head projections of h
    attn  = causal-softmax(q k^T / sqrt(HS)) v      (concat heads)
    x1 = x + attn @ Wproj + bproj
    h2 = LN(x1; g2, beta2)
    out = x1 + relu(h2 @ W1 + b1) @ W2 + b2

Sharding: batch 128 -> 16 sequences per core; all parameters replicated.

Design notes:
  * All matmul operands fp16 (PE 1 cycle/row, 10-bit mantissa), fp32 PSUM
    accumulation; activations/residuals fp32.
  * LN affine (g, beta) folded into the following weights on the host:
    W' = diag(g) W.  beta@W rows / biases are added via rank-1 (ones x row)
    matmuls accumulated in PSUM — emitted only when the row is nonzero
    (they are all zero for this problem's inputs, so they vanish).
  * Q/K stored head-transposed [d, t], heads padded to 64 rows so each
    128-row tile holds two heads (offsets 0/64); the two heads' score
    matmuls use distinct PE row groups and run concurrently.
  * Scores computed transposed [s, t]: softmax denominators reduce over
    the partition dim via a matmul against an appended ones-column of V
    (V stored augmented: 8 x (48 + 1) = 392 cols, ones via memset).
  * Causal mask applied post-exp as a 0/1 fp16 multiply on the two
    diagonal [128,128] blocks only; the all-masked block is never computed.
  * [t,c] <-> [c,t] layout changes bounce through DRAM scratch with one
    giant xbar-transpose DMA per c-chunk (DRAM source lifts the 128-row
    source limit), instead of 96 tiny SBUF-SBUF transposes that would
    serialize on the Sync engine.
  * Plain DMA on GpSimd SWDGE queues; only transposes use HWDGE (sync).
"""

import numpy as np

import concourse.bass as bass
import concourse.mybir as mybir
import concourse.tile as tile
from concourse import bacc
from concourse.bass_utils import run_bass_kernel_spmd

F32 = mybir.dt.float32
F16 = mybir.dt.float16

# Model dims
B, T, C = 128, 256, 384
H, HS = 8, 48
FF = 4 * C           # 1536
EPS = 1e-5

# Sharding / tiling
NCORES = 8
NB = B // NCORES     # 16 sequences per core
TOK = NB * T         # 4096 tokens per core
P = 128
CCH = C // P         # 3 c-chunks
FCH = FF // P        # 12 ffn chunks
DPAD = 512           # q/k head-padded dim (4 tiles x 2 heads x 64)
QMT = DPAD // P      # 4
VW = H * (HS + 1)    # 392 augmented v width
GT = 512             # tokens per group (2 sequences)
NG = TOK // GT       # 8 groups
ISCALE = float(HS) ** -0.5


def _build_program(flags):
    nc = bacc.Bacc(None, target_bir_lowering=False, debug=False)

    x_d = nc.dram_tensor("x", [TOK, C], F32, kind="ExternalInput").ap()
    wq_d = nc.dram_tensor("wq", [CCH, P, DPAD], F16, kind="ExternalInput").ap()
    wk_d = nc.dram_tensor("wk", [CCH, P, DPAD], F16, kind="ExternalInput").ap()
    wv_d = nc.dram_tensor("wv", [CCH, P, C], F16, kind="ExternalInput").ap()
    wp_d = nc.dram_tensor("wp", [CCH, P, C], F16, kind="ExternalInput").ap()
    w1_d = nc.dram_tensor("w1", [CCH, P, FF], F16, kind="ExternalInput").ap()
    w2_d = nc.dram_tensor("w2", [FCH, P, C], F16, kind="ExternalInput").ap()
    rowq_d = nc.dram_tensor("rowq", [1, DPAD], F16, kind="ExternalInput").ap()
    rowk_d = nc.dram_tensor("rowk", [1, DPAD], F16, kind="ExternalInput").ap()
    rowv_d = nc.dram_tensor("rowv", [1, C], F16, kind="ExternalInput").ap()
    rowp_d = nc.dram_tensor("rowp", [1, C], F16, kind="ExternalInput").ap()
    rowl_d = nc.dram_tensor("rowl", [1, C], F16, kind="ExternalInput").ap()
    b1t_d = nc.dram_tensor("b1t", [P, FCH], F32, kind="ExternalInput").ap()
    mask_d = nc.dram_tensor("maskmul", [P, P], F16, kind="ExternalInput").ap()
    out_d = nc.dram_tensor("out", [TOK, C], F32, kind="ExternalOutput").ap()
    # DRAM scratch for layout bounces
    h_scr = nc.dram_tensor("h_scr", [TOK, C], F16).ap()
    a_scr = nc.dram_tensor("a_scr", [TOK, C], F16).ap()
    h2_scr = nc.dram_tensor("h2_scr", [TOK, C], F16).ap()

    with tile.TileContext(nc) as tc:
        _emit(nc, tc, flags, x_d, wq_d, wk_d, wv_d, wp_d, w1_d, w2_d,
              rowq_d, rowk_d, rowv_d, rowp_d, rowl_d, b1t_d, mask_d, out_d,
              h_scr, a_scr, h2_scr)
    nc.compile()
    return nc


def _emit(nc, tc, flags, x_d, wq_d, wk_d, wv_d, wp_d, w1_d, w2_d,
          rowq_d, rowk_d, rowv_d, rowp_d, rowl_d, b1t_d, mask_d, out_d,
          h_scr, a_scr, h2_scr):
    from contextlib import ExitStack
    with ExitStack() as ctx:
        const = ctx.enter_context(tc.tile_pool(name="const", bufs=1))
        big = ctx.enter_context(tc.tile_pool(name="big", bufs=1))
        ln = ctx.enter_context(tc.tile_pool(name="ln", bufs=4))
        grp = ctx.enter_context(tc.tile_pool(name="grp", bufs=2))
        att = ctx.enter_context(tc.tile_pool(name="att", bufs=4))
        outp = ctx.enter_context(tc.tile_pool(name="outp", bufs=3))
        psum = ctx.enter_context(tc.tile_pool(name="psum", bufs=8, space="PSUM"))

        def ps_tile():
            return psum.tile([P, 512], F32, name="ps", tag="ps")

        # ---- constants ----
        wq_sb = const.tile([P, CCH, DPAD], F16)
        wk_sb = const.tile([P, CCH, DPAD], F16)
        wv_sb = const.tile([P, CCH, C], F16)
        wp_sb = const.tile([P, CCH, C], F16)
        w1_sb = const.tile([P, CCH, FF], F16)
        w2_sb = const.tile([P, FCH, C], F16)
        for cc in range(CCH):
            nc.gpsimd.dma_start(wq_sb[:, cc, :], wq_d[cc])
            nc.gpsimd.dma_start(wk_sb[:, cc, :], wk_d[cc])
            nc.gpsimd.dma_start(wv_sb[:, cc, :], wv_d[cc])
            nc.gpsimd.dma_start(wp_sb[:, cc, :], wp_d[cc])
            nc.gpsimd.dma_start(w1_sb[:, cc, :], w1_d[cc])
        for fc in range(FCH):
            nc.gpsimd.dma_start(w2_sb[:, fc, :], w2_d[fc])
        mask_sb = const.tile([P, P], F16)
        eps_sb = const.tile([P, 1], F32)
        nc.gpsimd.dma_start(mask_sb, mask_d)
        nc.vector.memset(eps_sb, EPS)

        ones_sb = const.tile([1, GT], F16)
        nc.vector.memset(ones_sb, 1.0)
        rowq_sb = const.tile([1, DPAD], F16)
        rowk_sb = const.tile([1, DPAD], F16)
        rowv_sb = const.tile([1, C], F16)
        rowp_sb = const.tile([1, C], F16)
        rowl_sb = const.tile([1, C], F16)
        b1t_sb = const.tile([P, FCH], F32)
        if flags["rowq"]:
            nc.gpsimd.dma_start(rowq_sb, rowq_d)
        if flags["rowk"]:
            nc.gpsimd.dma_start(rowk_sb, rowk_d)
        if flags["rowv"]:
            nc.gpsimd.dma_start(rowv_sb, rowv_d)
        if flags["rowp"]:
            nc.gpsimd.dma_start(rowp_sb, rowp_d)
        if flags["rowl"]:
            nc.gpsimd.dma_start(rowl_sb, rowl_d)
        if flags["b1t"]:
            nc.gpsimd.dma_start(b1t_sb, b1t_d)

        # ---- residual stream (fp32, resident) and transposed LN1 output ----
        x_all = big.tile([P, TOK // P, C], F32)
        hT = big.tile([P, CCH, TOK], F16)
        h2T = big.tile([P, CCH, TOK], F16)

        def layernorm_tile(src, dst_f16):
            """dst_f16 <- (src - mean) * rsqrt(var + eps), rows = tokens."""
            stats = ln.tile([P, 6], F32, tag="stats")
            mv = ln.tile([P, 2], F32, tag="mv")
            rstd = ln.tile([P, 1], F32, tag="rstd")
            nc.vector.bn_stats(out=stats, in_=src)
            nc.vector.bn_aggr(out=mv, in_=stats)
            nc.scalar.activation(out=rstd, in_=mv[:, 1:2],
                                 func=mybir.ActivationFunctionType.Sqrt,
                                 bias=eps_sb, scale=1.0)
            nc.vector.reciprocal(out=rstd, in_=rstd)
            nc.vector.tensor_scalar(out=dst_f16, in0=src,
                                    scalar1=mv[:, 0:1], scalar2=rstd,
                                    op0=mybir.AluOpType.subtract,
                                    op1=mybir.AluOpType.mult)

        # ---- phase 1: load x, LN1 -> h_scr (DRAM), giant transposes -> hT ----
        for it in range(TOK // P):
            nc.gpsimd.dma_start(x_all[:, it, :], x_d[it * P:(it + 1) * P, :])
            hN = ln.tile([P, C], F16, tag="hN")
            layernorm_tile(x_all[:, it, :], hN)
            nc.gpsimd.dma_start(h_scr[it * P:(it + 1) * P, :], hN)

        # ---- phases 2-4 per group of 2 sequences ----
        for g in range(NG):
            t0 = g * GT
            for cc in range(CCH):
                nc.sync.dma_start(hT[:, cc, t0:t0 + GT],
                                  h_scr[t0:t0 + GT, cc * P:(cc + 1) * P],
                                  transpose=True)
            qT = grp.tile([P, QMT, GT], F16, tag="qT")
            kT = grp.tile([P, QMT, GT], F16, tag="kT")
            vaug = grp.tile([P, GT // P, VW], F16, tag="vaug")
            attnT = grp.tile([P, CCH, GT], F16, tag="attnT")

            # Q/K projections, head-transposed+padded
            for dst, w_sb, row_sb, rowf in ((qT, wq_sb, rowq_sb, flags["rowq"]),
                                            (kT, wk_sb, rowk_sb, flags["rowk"])):
                for m in range(QMT):
                    ps = ps_tile()
                    for cc in range(CCH):
                        nc.tensor.matmul(ps, lhsT=w_sb[:, cc, m * P:(m + 1) * P],
                                         rhs=hT[:, cc, t0:t0 + GT],
                                         start=(cc == 0),
                                         stop=(cc == CCH - 1 and not rowf))
                    if rowf:
                        nc.tensor.matmul(ps, lhsT=row_sb[:, m * P:(m + 1) * P],
                                         rhs=ones_sb, start=False, stop=True)
                    nc.vector.tensor_copy(dst[:, m, :], ps)

            # V (natural layout, augmented with a ones column per head)
            for st in range(GT // P):
                ps = ps_tile()
                for cc in range(CCH):
                    nc.tensor.matmul(ps[:, :C],
                                     lhsT=hT[:, cc, t0 + st * P:t0 + (st + 1) * P],
                                     rhs=wv_sb[:, cc, :],
                                     start=(cc == 0),
                                     stop=(cc == CCH - 1 and not flags["rowv"]))
                if flags["rowv"]:
                    nc.tensor.matmul(ps[:, :C], lhsT=ones_sb[:, :P], rhs=rowv_sb,
                                     start=False, stop=True)
                v3 = vaug[:, st, :].rearrange("p (h w) -> p h w", w=HS + 1)
                nc.vector.tensor_copy(
                    v3[:, :, :HS],
                    ps[:, :C].rearrange("p (h w) -> p h w", w=HS))
                nc.vector.memset(v3[:, :, HS], 1.0)

            # attention per sequence in group
            for b2 in range(2):
                s0 = b2 * T   # token offset of this seq inside the group
                aps0 = ps_tile()   # attn numerator+denominator, t-chunk 0
                aps1 = ps_tile()   # t-chunk 1
                for m in range(QMT):
                    for hh in range(2):
                        h = 2 * m + hh
                        off = 64 * hh
                        # transposed scores [s, t] for this head
                        sp0 = ps_tile()
                        nc.tensor.matmul(
                            sp0[:, :T],
                            lhsT=kT[off:off + HS, m, s0:s0 + P],
                            rhs=qT[off:off + HS, m, s0:s0 + T],
                            start=True, stop=True)
                        sp1 = ps_tile()
                        nc.tensor.matmul(
                            sp1[:, :P],
                            lhsT=kT[off:off + HS, m, s0 + P:s0 + T],
                            rhs=qT[off:off + HS, m, s0 + P:s0 + T],
                            start=True, stop=True)
                        ew0 = att.tile([P, T], F16, tag="ew0")
                        ew1 = att.tile([P, P], F16, tag="ew1")
                        nc.scalar.activation(out=ew0, in_=sp0[:, :T],
                                             func=mybir.ActivationFunctionType.Exp,
                                             scale=ISCALE)
                        nc.scalar.activation(out=ew1, in_=sp1[:, :P],
                                             func=mybir.ActivationFunctionType.Exp,
                                             scale=ISCALE)
                        # causal mask on the two diagonal blocks
                        nc.vector.tensor_mul(ew0[:, :P], ew0[:, :P], mask_sb)
                        nc.vector.tensor_mul(ew1, ew1, mask_sb)
                        hs = h * (HS + 1)
                        # t-chunk 0 only sees s-chunk 0
                        nc.tensor.matmul(aps0[:, hs:hs + HS + 1],
                                         lhsT=ew0[:, :P],
                                         rhs=vaug[:, b2 * 2, hs:hs + HS + 1],
                                         start=True, stop=True)
                        # t-chunk 1 sees both s-chunks
                        nc.tensor.matmul(aps1[:, hs:hs + HS + 1],
                                         lhsT=ew0[:, P:T],
                                         rhs=vaug[:, b2 * 2, hs:hs + HS + 1],
                                         start=True, stop=False)
                        nc.tensor.matmul(aps1[:, hs:hs + HS + 1],
                                         lhsT=ew1,
                                         rhs=vaug[:, b2 * 2 + 1, hs:hs + HS + 1],
                                         start=False, stop=True)
                # normalize; write natural-layout attn to DRAM scratch
                for tch, aps in ((0, aps0), (1, aps1)):
                    a3 = aps[:, :VW].rearrange("p (h w) -> p h w", w=HS + 1)
                    recips = att.tile([P, H], F32, tag="recips")
                    nc.vector.reciprocal(out=recips, in_=a3[:, :, HS])
                    attn_n = att.tile([P, C], F16, tag="attn_n")
                    for h in range(H):
                        nc.vector.tensor_scalar_mul(
                            out=attn_n[:, h * HS:(h + 1) * HS],
                            in0=a3[:, h, :HS], scalar1=recips[:, h:h + 1])
                    tt = b2 * 2 + tch
                    row0 = t0 + tt * P
                    nc.gpsimd.dma_start(a_scr[row0:row0 + P, :], attn_n)
            for cc in range(CCH):
                nc.sync.dma_start(attnT[:, cc, :],
                                  a_scr[t0:t0 + GT, cc * P:(cc + 1) * P],
                                  transpose=True)

            # output projection + residual (x1 overwrites x in place)
            for tt in range(GT // P):
                it = g * (GT // P) + tt
                ps = ps_tile()
                for cc in range(CCH):
                    nc.tensor.matmul(ps[:, :C],
                                     lhsT=attnT[:, cc, tt * P:(tt + 1) * P],
                                     rhs=wp_sb[:, cc, :],
                                     start=(cc == 0),
                                     stop=(cc == CCH - 1 and not flags["rowp"]))
                if flags["rowp"]:
                    nc.tensor.matmul(ps[:, :C], lhsT=ones_sb[:, :P], rhs=rowp_sb,
                                     start=False, stop=True)
                nc.vector.tensor_add(x_all[:, it, :], x_all[:, it, :], ps[:, :C])

        # ---- phase 5: LN2 over all tokens -> h2_scr -> giant transposes ----
        for it in range(TOK // P):
            h2 = ln.tile([P, C], F16, tag="h2")
            layernorm_tile(x_all[:, it, :], h2)
            nc.gpsimd.dma_start(h2_scr[it * P:(it + 1) * P, :], h2)

        # ---- phases 6-7 per group: FFN + residual ----
        for g in range(NG):
            t0 = g * GT
            for cc in range(CCH):
                nc.sync.dma_start(h2T[:, cc, t0:t0 + GT],
                                  h2_scr[t0:t0 + GT, cc * P:(cc + 1) * P],
                                  transpose=True)
            rg = grp.tile([P, FCH, GT], F16, tag="rg")
            for fc in range(FCH):
                ps = ps_tile()
                for cc in range(CCH):
                    nc.tensor.matmul(ps, lhsT=w1_sb[:, cc, fc * P:(fc + 1) * P],
                                     rhs=h2T[:, cc, t0:t0 + GT],
                                     start=(cc == 0), stop=(cc == CCH - 1))
                bias = b1t_sb[:, fc:fc + 1] if flags["b1t"] else 0.0
                nc.scalar.activation(out=rg[:, fc, :], in_=ps,
                                     func=mybir.ActivationFunctionType.Relu,
                                     bias=bias, scale=1.0)
            for tt in range(GT // P):
                it = g * (GT // P) + tt
                ps = ps_tile()
                for fc in range(FCH):
                    nc.tensor.matmul(ps[:, :C],
                                     lhsT=rg[:, fc, tt * P:(tt + 1) * P],
                                     rhs=w2_sb[:, fc, :],
                                     start=(fc == 0),
                                     stop=(fc == FCH - 1 and not flags["rowl"]))
                if flags["rowl"]:
                    nc.tensor.matmul(ps[:, :C], lhsT=ones_sb[:, :P], rhs=rowl_sb,
                                     start=False, stop=True)
                ot = outp.tile([P, C], F32, tag="ot")
                nc.vector.tensor_add(ot, x_all[:, it, :], ps[:, :C])
                nc.gpsimd.dma_start(out_d[it * P:(it + 1) * P, :], ot)


def _prep_weights(Wq, Wk, Wv, Wproj, bproj, W1, b1, W2, b2, g1, beta1, g2, beta2):
    f16 = np.float16
    g1 = g1.astype(np.float64)
    g2 = g2.astype(np.float64)

    def qk_pack(W):
        Ws = g1[None, :, None] * W.astype(np.float64)      # [H, C, HS]
        pad = np.zeros((CCH, P, DPAD), np.float64)
        row = np.zeros((1, DPAD), np.float64)
        # beta1 @ W uses the unscaled W: h_aff@W = h_norm@(g1*W) + beta1@W
        beta_r = np.einsum('c,hcd->hd', beta1.astype(np.float64),
                           W.astype(np.float64))
        for h in range(H):
            m, hh = divmod(h, 2)
            col = m * P + 64 * hh
            pad[:, :, col:col + HS] = Ws[h].reshape(CCH, P, HS)
            row[0, col:col + HS] = beta_r[h]
        return pad.astype(f16), row.astype(f16)

    wq_pad, rowq = qk_pack(Wq)
    wk_pad, rowk = qk_pack(Wk)

    # V: plain concat-head layout [C, C]; ones column added on-chip
    Wvs = (g1[None, :, None] * Wv.astype(np.float64))       # [H, C, HS]
    wv = np.transpose(Wvs, (1, 0, 2)).reshape(C, C)         # [c, h*HS+d]
    beta_v = np.einsum('c,hcd->hd', beta1.astype(np.float64),
                       Wv.astype(np.float64)).reshape(1, C)
    wv = wv.astype(f16).reshape(CCH, P, C)
    rowv = beta_v.astype(f16)

    wp = Wproj.astype(f16).reshape(CCH, P, C)
    rowp = bproj.astype(f16).reshape(1, C)

    W1s = g2[:, None] * W1.astype(np.float64)
    w1p = W1s.astype(f16).reshape(CCH, P, FF)
    b1tot = (b1.astype(np.float64)
             + beta2.astype(np.float64) @ W1.astype(np.float64))
    b1t = b1tot.astype(np.float32).reshape(FCH, P).T.copy()   # [P, FCH]

    w2p = W2.astype(f16).reshape(FCH, P, C)
    rowl = b2.astype(f16).reshape(1, C)

    maskmul = np.triu(np.ones((P, P), f16))  # [s, t]: valid iff s <= t
    wdict = dict(wq=wq_pad, wk=wk_pad, wv=wv, wp=wp, w1=w1p, w2=w2p,
                 rowq=rowq, rowk=rowk, rowv=rowv, rowp=rowp, rowl=rowl,
                 b1t=b1t, maskmul=maskmul)
    flags = {k: bool(np.any(wdict[k] != 0))
             for k in ("rowq", "rowk", "rowv", "rowp", "rowl", "b1t")}
    return wdict, flags


_CACHED = {}


def _get_program(flags):
    key = tuple(sorted(flags.items()))
    if key not in _CACHED:
        _CACHED[key] = _build_program(flags)
    return _CACHED[key]


def _run(inputs, trace=False):
    x = np.asarray(inputs["x"], np.float32)
    wdict, flags = _prep_weights(
        np.asarray(inputs["Wq"]), np.asarray(inputs["Wk"]),
        np.asarray(inputs["Wv"]), np.asarray(inputs["Wproj"]),
        np.asarray(inputs["bproj"]), np.asarray(inputs["W1"]),
        np.asarray(inputs["b1"]), np.asarray(inputs["W2"]),
        np.asarray(inputs["b2"]), np.asarray(inputs["g1"]),
        np.asarray(inputs["beta1"]), np.asarray(inputs["g2"]),
        np.asarray(inputs["beta2"]))

    shards = x.reshape(NCORES, NB * T, C)
    in_maps = [dict(wdict, x=np.ascontiguousarray(shards[i]))
               for i in range(NCORES)]
    nc = _get_program(flags)
    res = run_bass_kernel_spmd(nc, in_maps, list(range(NCORES)), trace=trace)
    out = np.stack([res.results[i]["out"] for i in range(NCORES)])
    return out.reshape(B, T, C).astype(np.float32), res


def kernel(**inputs):
    out, _ = _run(inputs, trace=False)
    return out
